# revision 14
# baseline (speedup 1.0000x reference)
"""Trainium2 Bass kernel for nn_Attention_90787018703157 (sparse_attention).

Reference computation (per batch element b):
    q = s @ Wq.T                      # [N, 32]
    k = s @ Wk.T                      # [N, 32]
    logits = q @ k.T                  # [N, N]
    w = logits**2 * G
    out = w / (w.sum(-1, keepdims=True) + 1e-6)

Sharding: data-parallel over the batch dim — B=8 batch elements, one per
NeuronCore.  Wq/Wk are replicated.

Precision: the harness gate is rel_err < 2e-2; the elementwise path
(square, G-mult, normalize) tolerates 16-bit easily (~1e-3 rel err), so
G is cast to fp16 on the host and the output is produced as fp16 on
chip (re-expanded to fp32 on the host).  That halves the HBM traffic:
8.39 MiB in + 8.39 MiB out per core -> ~47 us floor at 358 GB/s,
vs ~94 us for the fp32 version.  q/k/logits stay fp32 on the PE.

DMA schedule (variant A): G and out are both fully SBUF-resident
(8 MiB each).  All transfers ride ONE HWDGE ring (SP) in program order:
the 4 G loads are issued first, the 4 out stores queue up behind them in
the ring FIFO as compute completes.  Reads and writes therefore never
interleave on HBM (no read/write turnaround), and each 2 MiB transfer
is split across all 16 SDMA engines at full rate.

Per-core compute (per 128-row block t):
    logits_ps[128, 2048] = qT_blk.T @ kT   (4 fp32 matmuls, K=32)
    sq   = Square(logits_ps) -> fp16       (ScalarE, PSUM->SBUF)
    o,rs = sq * G_blk, rowsum fused        (VectorE stt, fp16 2x mode)
    rc   = 1/(rs + 1e-6)                   (VectorE, fp32 [128,1])
    o   *= rc  in place                    (VectorE tensor_scalar, 4x mode)
Engine budgets/block: DVE ~2.0us, ACT ~1.9us, PE ~1.3us -> all well
under the 2.9us/block DMA floor.
"""

from contextlib import ExitStack

import numpy as np

import concourse.bass as bass
import concourse.bacc as bacc
import concourse.tile as tile
from concourse import mybir
from concourse.bass_utils import run_bass_kernel_spmd
from concourse.masks import make_identity

B = 8
N = 2048
IN_DIM = 10
QK = 32
P = 128
NT = N // P      # 16 row blocks per core
MB = 512         # max moving free dim for fp32 matmul
NMB = N // MB    # 4
F32 = mybir.dt.float32
F16 = mybir.dt.float16
EPS = 1e-6

# DMA variant: "phase" = single-ring phase-separated (loads then stores on
# the SP HWDGE ring, G/out fully SBUF-resident); "mixed" = baseline-style
# (loads alternate SP/ACT rings, stores on SWDGE, double-buffered).
# Env overrides are for A/B experiments only; defaults are the shipping
# config.
import os as _os

VARIANT = _os.environ.get("BASS_VARIANT", "phase")
BPD = int(_os.environ.get("BASS_BPD", "4"))  # row blocks per DMA chunk
# Ablation for timing experiments only: "dma" = loads+stores without
# compute; "compute" = compute without G loads/stores.
ABLATE = _os.environ.get("BASS_ABLATE", "none")


def _build_nc(
    loop_reps: int = 1,
    hw_loop: bool = False,
    unroll: int = 1,
    io_internal: bool = False,
) -> bass.Bass:
    # Bacc (not plain Bass): its finalize() runs move_matmul_waits_to_ldweights
    # + generate_event_semaphores, which split multi-wait instructions to
    # satisfy the TRN2 one-wait-per-instruction constraint.
    #
    # io_internal=True is the timing build: G/out become Internal DRAM
    # scratch (garbage values; timing of this kernel is data-independent)
    # so each axon-proxied run only ships ~100 KB instead of ~135 MB, and
    # a tiny dummy output keeps PJRT happy.
    nc = bacc.Bacc()

    big_in = "Internal" if io_internal else "ExternalInput"
    big_out = "Internal" if io_internal else "ExternalOutput"
    s_d = nc.dram_tensor("s", [N, IN_DIM], F32, kind="ExternalInput")
    G_d = nc.dram_tensor("G", [N, N], F16, kind=big_in)
    wq_d = nc.dram_tensor("Wq", [QK, IN_DIM], F32, kind="ExternalInput")
    wk_d = nc.dram_tensor("Wk", [QK, IN_DIM], F32, kind="ExternalInput")
    out_d = nc.dram_tensor("out", [N, N], F16, kind=big_out)
    tick_d = (
        nc.dram_tensor("tick", [P, 4], F32, kind="ExternalOutput")
        if io_internal
        else None
    )

    with tile.TileContext(nc) as tc, ExitStack() as ctx:
        consts = ctx.enter_context(tc.tile_pool(name="consts", bufs=1))

        ident = consts.tile([P, P], F32)
        make_identity(nc, ident)

        wqT = consts.tile([IN_DIM, QK], F32)
        nc.sync.dma_start(out=wqT, in_=wq_d.rearrange("q i -> i q"))
        wkT = consts.tile([IN_DIM, QK], F32)
        nc.sync.dma_start(out=wkT, in_=wk_d.rearrange("q i -> i q"))

        # s loaded so that row-block t sits at free-dim slot t: [128, 16, 10]
        s_sb = consts.tile([P, NT, IN_DIM], F32)
        s_v = s_d.rearrange("(t p) i -> p t i", p=P)
        nc.sync.dma_start(out=s_sb, in_=s_v)

        sT = consts.tile([IN_DIM, N], F32)
        # fp16 q/k: fp32 matmuls run at 4 cycles/row on the PE, fp16 at 1.
        # fp16's 10 mantissa bits keep the logits error ~1e-3 relative.
        qT = consts.tile([QK, N], F16)
        kT = consts.tile([QK, N], F16)

        NU = NT // BPD
        G_v = G_d.rearrange("(u b p) m -> u p b m", p=P, b=BPD)
        o_v = out_d.rearrange("(u b p) m -> u p b m", p=P, b=BPD)

        if VARIANT == "phase":
            g_sb = consts.tile([P, NT, N], F16)
            o_sb = consts.tile([P, NT, N], F16)
            if ABLATE == "dma":
                nc.vector.memset(o_sb, 0.0)
            elif ABLATE == "compute":
                nc.vector.memset(g_sb, 0.0)
        else:
            g_pool = ctx.enter_context(tc.tile_pool(name="g", bufs=4))
            o_pool = ctx.enter_context(tc.tile_pool(name="o", bufs=3))

        # Preamble runs once (outside any timing loop): compute qT/kT.
        # Its PSUM pool must close before ps_pool below takes all 8 banks.
        # Per 512-col m-block: 4 PE transposes -> sT slice -> q/k
        # projection matmuls -> SBUF.
        with tc.tile_pool(name="pre_ps", bufs=2, space="PSUM") as pre_ps:
            for m in range(NMB):
                sl = slice(m * MB, (m + 1) * MB)
                tr_ps = pre_ps.tile([IN_DIM, MB], F32, tag="tr", name="tr_ps")
                for j in range(4):
                    t = 4 * m + j
                    nc.tensor.transpose(
                        tr_ps[:, j * P : (j + 1) * P], s_sb[:, t, :], ident
                    )
                nc.scalar.copy(sT[:, sl], tr_ps)
                q_ps = pre_ps.tile([QK, MB], F32, tag="qps", name="q_ps")
                nc.tensor.matmul(q_ps, wqT, sT[:, sl])
                nc.vector.tensor_copy(qT[:, sl], q_ps)
                k_ps = pre_ps.tile([QK, MB], F32, tag="kps", name="k_ps")
                nc.tensor.matmul(k_ps, wkT, sT[:, sl])
                nc.scalar.copy(kT[:, sl], k_ps)

        sq_pool = ctx.enter_context(tc.tile_pool(name="sq", bufs=3))
        small = ctx.enter_context(tc.tile_pool(name="small", bufs=4))
        ps_pool = ctx.enter_context(tc.tile_pool(name="ps", bufs=2, space="PSUM"))

        def block_compute(t, g_ap, o_ap):
            # g_ap/o_ap: [P, N] fp16 views for row-block t
            lg = ps_pool.tile([P, N], F32, name="lg")
            for m in range(NMB):
                sl = slice(m * MB, (m + 1) * MB)
                nc.tensor.matmul(lg[:, sl], qT[:, t * P : (t + 1) * P], kT[:, sl])

            sq_t = sq_pool.tile([P, N], F16, name="sq_t")
            nc.scalar.activation(sq_t, lg, mybir.ActivationFunctionType.Square)

            # w = sq * G written straight into the output tile,
            # rs = rowsum(w) fused in (InstTensorScalarPtr)
            rs = small.tile([P, 1], F32, tag="rs", name="rs")
            nc.vector.scalar_tensor_tensor(
                out=o_ap,
                in0=sq_t,
                scalar=1.0,
                in1=g_ap,
                op0=mybir.AluOpType.mult,
                op1=mybir.AluOpType.mult,
                accum_out=rs,
            )
            rse = small.tile([P, 1], F32, tag="rse", name="rse")
            nc.vector.tensor_scalar_add(rse, rs, EPS)
            rc = small.tile([P, 1], F32, tag="rc", name="rc")
            nc.vector.reciprocal(rc, rse)
            nc.vector.tensor_scalar_mul(o_ap, o_ap, rc)

        def one_pass_phase():
            # "phase": every transfer on the SP ring in program order —
            # loads first, stores behind them in the ring FIFO, so HBM sees
            # a pure read phase then a pure write phase.
            # "phase2": loads alternate SP/ACT rings (two rings hide each
            # other's completion-receipt gaps); stores split SP/SWDGE.
            # Store waits never sit on the ACT queue (would stall Squares).
            two_ring = VARIANT == "phase2"
            if ABLATE != "compute":
                for u in range(NU):
                    eng = nc.scalar if (two_ring and u % 2 == 1) else nc.sync
                    eng.dma_start(
                        out=g_sb[:, u * BPD : (u + 1) * BPD, :], in_=G_v[u]
                    )
            for u in range(NU):
                if ABLATE != "dma":
                    for b in range(BPD):
                        t = BPD * u + b
                        block_compute(t, g_sb[:, t, :], o_sb[:, t, :])
                if ABLATE != "compute":
                    eng = nc.gpsimd if (two_ring and u % 2 == 1) else nc.sync
                    eng.dma_start(
                        out=o_v[u], in_=o_sb[:, u * BPD : (u + 1) * BPD, :]
                    )

        def one_pass_mixed():
            for u in range(NU):
                g2 = g_pool.tile([P, BPD, N], F16, name="g2")
                (nc.sync if u % 2 == 0 else nc.scalar).dma_start(out=g2, in_=G_v[u])
                o2 = o_pool.tile([P, BPD, N], F16, name="o2")
                for b in range(BPD):
                    t = BPD * u + b
                    block_compute(t, g2[:, b, :], o2[:, b, :])
                nc.gpsimd.dma_start(out=o_v[u], in_=o2)

        one_pass_inner = one_pass_phase if VARIANT == "phase" else one_pass_mixed

        def one_pass():
            for _ in range(unroll):
                one_pass_inner()
            if tick_d is not None:
                nc.sync.dma_start(out=tick_d[:, :], in_=s_sb[:, 0, 0:4])

        if hw_loop and loop_reps > 1:
            with tc.For_i(0, loop_reps, 1):
                one_pass()
        else:
            for _ in range(loop_reps):
                one_pass()

    nc.finalize()
    return nc


_NC_CACHE = {}


def _get_nc(
    loop_reps: int = 1,
    hw_loop: bool = False,
    unroll: int = 1,
    io_internal: bool = False,
) -> bass.Bass:
    key = (loop_reps, hw_loop, unroll, io_internal)
    if key not in _NC_CACHE:
        _NC_CACHE[key] = _build_nc(loop_reps, hw_loop, unroll, io_internal)
    return _NC_CACHE[key]


def _in_maps(inputs):
    s = np.ascontiguousarray(np.asarray(inputs["s"], dtype=np.float32))
    G = np.ascontiguousarray(np.asarray(inputs["G"], dtype=np.float16))
    Wq = np.ascontiguousarray(np.asarray(inputs["Wq"], dtype=np.float32))
    Wk = np.ascontiguousarray(np.asarray(inputs["Wk"], dtype=np.float32))
    assert s.shape == (B, N, IN_DIM), s.shape
    assert G.shape == (B, N, N), G.shape
    return [{"s": s[b], "G": G[b], "Wq": Wq, "Wk": Wk} for b in range(B)]


def _run(inputs, trace: bool = False):
    nc = _get_nc()
    in_maps = _in_maps(inputs)
    res = run_bass_kernel_spmd(nc, in_maps, core_ids=list(range(B)), trace=trace)
    out = np.stack(
        [res.results[b]["out"].astype(np.float32) for b in range(B)], axis=0
    )
    return out, res


def kernel(s, G, Wq, Wk):
    out, _ = _run({"s": s, "G": G, "Wq": Wq, "Wk": Wk})
    return out
